# revision 1
# baseline (speedup 1.0000x reference)
# Trainium2 Bass kernel for nn_BinaryClassifier (one-hot -> LSTM -> FC).
#
# Strategy: data-parallel over batch, 8 sequences per NeuronCore, all 8
# cores run the same program on different shards. Per core the LSTM
# recurrence runs 2048 sequential steps; each step streams W_hh through
# the PE as 64 bf16 [128,128] stationary tiles (fast-weight-load) against
# the h state kept transposed ([128 hidden, 8 batch] columns), gates land
# in PSUM [128, 32] per hidden slice, activations on ACT/DVE. The input
# projection is an embedding lookup: E = W_ih.T + b_ih + b_hh gathered by
# token; it is precomputed on the PE in 16-step chunks via K=25 matmuls
# with host-built one-hot columns as the moving operand (double-buffered),
# then added to the gates on DVE. h at t = len-1 is captured with an
# is_equal mask; the final FC runs on-device.
import sys
sys.path.insert(0, '/opt/trn_rl_repo')
from contextlib import ExitStack

import numpy as np
import ml_dtypes

import concourse.bass as bass
import concourse.mybir as mybir
from concourse.tile import TileContext
from concourse.bass import ds
from concourse.bass_utils import run_bass_kernel_spmd

F32 = mybir.dt.float32
BF16 = mybir.dt.bfloat16
AF = mybir.ActivationFunctionType
ALU = mybir.AluOpType

H = 512
V = 25
B, S = 64, 2048
N_CORES = 8
BLOC = 8          # sequences per core
CH = 16           # steps per embedding chunk
BODY = 2 * CH     # steps per For_i body
NM = 16           # gate tiles (4H / 128)
NK = 4            # contraction tiles (H / 128)

_TPB_ENGINES = None


def _split_multi_waits(nc):
    """walrus in this container supports only ONE sync wait per TPB engine
    instruction; split extra waits onto preceding same-engine NOPs."""
    global _TPB_ENGINES
    if _TPB_ENGINES is None:
        _TPB_ENGINES = {mybir.EngineType.Pool, mybir.EngineType.Activation,
                        mybir.EngineType.PE, mybir.EngineType.DVE,
                        mybir.EngineType.SP}
    ctr = 0
    for fn in nc.m.functions:
        for bb in fn.blocks:
            new = []
            for inst in bb.instructions:
                si = inst.sync_info
                if (si is not None and len(si.on_wait) > 1
                        and inst.engine in _TPB_ENGINES):
                    waits = list(si.on_wait)
                    for w in waits[:-1]:
                        nop = mybir.InstNoOp(name=f"wsplit-{ctr}", ins=[],
                                             outs=[])
                        ctr += 1
                        nop.engine = inst.engine
                        nop.sync_info = mybir.SyncInfo(on_wait=[w],
                                                       on_update=[])
                        new.append(nop)
                    si.on_wait = waits[-1:]
                    inst.sync_info = si
                new.append(inst)
            bb.instructions = new


def _host_prep(tokens, lengths, W_ih, W_hh, b_ih, b_hh, fc_w, fc_b):
    """Full inputs -> per-core device input dicts.

    Gate-tile numbering: m = j*4 + g, j = hidden slice (0..3),
    g in {0:i, 1:f, 2:o, 3:g_cell} (reordered from torch i,f,g,o)."""
    bf = ml_dtypes.bfloat16
    lengths = np.asarray(lengths).astype(np.int64)
    order = np.argsort(-lengths, kind='stable')
    toks = np.asarray(tokens)[order]
    lens = lengths[order]

    perm = np.concatenate([np.arange(0 * H, 1 * H),      # i
                           np.arange(1 * H, 2 * H),      # f
                           np.arange(3 * H, 4 * H),      # o
                           np.arange(2 * H, 3 * H)])     # g_cell
    Whh_p = np.asarray(W_hh, np.float32)[perm]           # [4H, H]
    E_p = (np.asarray(W_ih, np.float32)
           + np.asarray(b_ih, np.float32)[:, None]
           + np.asarray(b_hh, np.float32)[:, None])[perm]

    w = np.zeros((128, NK * NM * 128), np.float32)
    e = np.zeros((V, NM * 128), np.float32)
    for j in range(4):
        for g in range(4):
            m = j * 4 + g
            rows = slice(g * H + j * 128, g * H + j * 128 + 128)
            for k in range(NK):
                blk = Whh_p[rows, k * 128:(k + 1) * 128]
                w[:, (k * NM + m) * 128:(k * NM + m + 1) * 128] = blk.T
            e[:, m * 128:(m + 1) * 128] = E_p[rows, :].T

    fcw = np.zeros((128, 4), np.float32)
    for j in range(4):
        fcw[:, j] = np.asarray(fc_w, np.float32)[0, j * 128:(j + 1) * 128]

    per_core = []
    for ci in range(N_CORES):
        bs = slice(ci * BLOC, (ci + 1) * BLOC)
        t_c = toks[bs]
        l_c = lens[bs]
        oh = np.zeros((V, S * BLOC + 2 * CH * BLOC), np.float32)
        sidx = np.arange(S)
        for b in range(BLOC):
            oh[t_c[b], sidx * BLOC + b] = 1.0
        lcap = np.tile((l_c - 1).astype(np.float32), 4)
        lcap = np.broadcast_to(lcap, (128, 32)).copy()
        fcb = np.full((BLOC, 1), np.asarray(fc_b, np.float32)[0], np.float32)
        per_core.append({
            "w_lhsT": w.astype(bf),
            "e_lhsT": e.astype(bf),
            "onehot": oh.astype(bf),
            "lcap": lcap,
            "fcw": fcw,
            "fcb": fcb,
        })
    return per_core, order


def _build_nc():
    ITERS = S // BODY
    nc = bass.Bass("TRN2", target_bir_lowering=False, debug=False,
                   num_devices=N_CORES)
    w_d = nc.dram_tensor("w_lhsT", [128, NK * NM * 128], BF16,
                         kind="ExternalInput").ap()
    e_d = nc.dram_tensor("e_lhsT", [V, NM * 128], BF16,
                         kind="ExternalInput").ap()
    oh_d = nc.dram_tensor("onehot", [V, S * BLOC + 2 * CH * BLOC], BF16,
                          kind="ExternalInput").ap()
    lcap_d = nc.dram_tensor("lcap", [128, 32], F32, kind="ExternalInput").ap()
    fcw_d = nc.dram_tensor("fcw", [128, 4], F32, kind="ExternalInput").ap()
    fcb_d = nc.dram_tensor("fcb", [BLOC, 1], F32, kind="ExternalInput").ap()
    out_d = nc.dram_tensor("out", [BLOC, 1], F32, kind="ExternalOutput").ap()

    with TileContext(nc) as tc, ExitStack() as ctx:
        const = ctx.enter_context(tc.tile_pool(name="const", bufs=1))
        state = ctx.enter_context(tc.tile_pool(name="state", bufs=1))
        scr = ctx.enter_context(tc.tile_pool(name="scr", bufs=3))
        ohp = ctx.enter_context(tc.tile_pool(name="ohp", bufs=2))

        w_sb = const.tile([128, NK * NM * 128], BF16, tag="w")
        e_sb = const.tile([V, NM * 128], BF16, tag="e")
        lcap = const.tile([128, 32], F32, tag="lcap")
        fcw = const.tile([128, 4], F32, tag="fcw")
        fcb = const.tile([BLOC, 1], F32, tag="fcb")
        nc.sync.dma_start(out=w_sb[:], in_=w_d[:])
        nc.sync.dma_start(out=e_sb[:], in_=e_d[:])
        nc.sync.dma_start(out=lcap[:], in_=lcap_d[:])
        nc.sync.dma_start(out=fcw[:], in_=fcw_d[:])
        nc.sync.dma_start(out=fcb[:], in_=fcb_d[:])

        # double-buffered transposed h state: matmuls of a step read the
        # full previous-step h, activations write the other buffer
        hTA = state.tile([128, 32], BF16, tag="hTA")     # col k*8+b
        hTB = state.tile([128, 32], BF16, tag="hTB")
        c_st = state.tile([128, 32], F32, tag="c")
        hfin = state.tile([128, 32], F32, tag="hfin")
        tcnt = state.tile([128, 32], F32, tag="tcnt")
        for t in (hTA, hTB, c_st, hfin, tcnt):
            nc.vector.memset(t[:], 0)

        # embedding-projection buffers: col sc*128 + m*8 + b (t-major)
        xgA = state.tile([128, CH * 128], F32, tag="xgA")
        xgB = state.tile([128, CH * 128], F32, tag="xgB")

        with tc.tile_pool(name="psum", bufs=1, space="PSUM") as psum:
            gp = [psum.tile([128, 32], F32, name=f"gp{j}", tag=f"gp{j}")
                  for j in range(4)]
            xgp = psum.tile([128, CH * 128], F32, tag="xgp")

            def produce_xg(oh_tile, col0, xg_sb):
                for m in range(NM):
                    # one accumulation group per 2KB psum zero region
                    nc.tensor.matmul(
                        xgp[:, m * CH * BLOC:(m + 1) * CH * BLOC],
                        e_sb[:, m * 128:(m + 1) * 128],
                        oh_tile[:, col0:col0 + CH * BLOC],
                        start=(m % 4 == 0), stop=(m % 4 == 3))
                src = xgp[:].rearrange("p (m t b) -> p t m b",
                                       m=NM, t=CH, b=BLOC)
                dst = xg_sb[:].rearrange("p (t m b) -> p t m b",
                                         m=NM, t=CH, b=BLOC)
                nc.scalar.copy(dst, src)

            def step(sc, xg_sb, hT, hTn):
                for j in range(4):
                    for k in range(NK):
                        for g in range(4):
                            m = j * 4 + g
                            # single accumulation group per slice per step
                            nc.tensor.matmul(
                                gp[j][:, g * 8:(g + 1) * 8],
                                w_sb[:, (k * NM + m) * 128:
                                     (k * NM + m + 1) * 128],
                                hT[:, k * 8:(k + 1) * 8],
                                start=(k == 0 and g == 0),
                                stop=(k == NK - 1 and g == 3))
                    gs = scr.tile([128, 32], F32, tag="gs")
                    nc.vector.tensor_tensor(
                        gs[:], gp[j][:],
                        xg_sb[:, sc * 128 + j * 32:sc * 128 + (j + 1) * 32],
                        op=ALU.add)
                    sact = scr.tile([128, 24], F32, tag="sact")
                    gact = scr.tile([128, 8], F32, tag="gact")
                    nc.scalar.activation(sact[:], gs[:, 0:24], AF.Sigmoid)
                    nc.scalar.activation(gact[:], gs[:, 24:32], AF.Tanh)
                    t1 = scr.tile([128, 8], F32, tag="t1")
                    t2 = scr.tile([128, 8], F32, tag="t2")
                    nc.vector.tensor_tensor(t1[:], sact[:, 0:8], gact[:],
                                            op=ALU.mult)
                    nc.vector.tensor_tensor(t2[:], sact[:, 8:16],
                                            c_st[:, j * 8:(j + 1) * 8],
                                            op=ALU.mult)
                    nc.vector.tensor_tensor(c_st[:, j * 8:(j + 1) * 8],
                                            t1[:], t2[:], op=ALU.add)
                    tnc = scr.tile([128, 8], F32, tag="tnc")
                    nc.scalar.activation(tnc[:], c_st[:, j * 8:(j + 1) * 8],
                                         AF.Tanh)
                    nc.vector.tensor_tensor(hTn[:, j * 8:(j + 1) * 8],
                                            sact[:, 16:24], tnc[:],
                                            op=ALU.mult)
                # capture h at t == len-1
                m32 = scr.tile([128, 32], F32, tag="m32")
                tmp = scr.tile([128, 32], F32, tag="tmp")
                nc.vector.tensor_tensor(m32[:], tcnt[:], lcap[:],
                                        op=ALU.is_equal)
                nc.vector.tensor_tensor(tmp[:], m32[:], hTn[:], op=ALU.mult)
                nc.vector.tensor_tensor(hfin[:], hfin[:], tmp[:], op=ALU.add)
                nc.vector.tensor_scalar_add(tcnt[:], tcnt[:], 1.0)

            oh0 = ohp.tile([V, 2 * CH * BLOC], BF16, tag="oh")
            nc.sync.dma_start(out=oh0[:], in_=oh_d[:, 0:2 * CH * BLOC])
            produce_xg(oh0, 0, xgA)
            produce_xg(oh0, CH * BLOC, xgB)

            with tc.For_i(0, ITERS, 1,
                          hint_engines=(mybir.EngineType.PE,)) as iv:
                oh = ohp.tile([V, 2 * CH * BLOC], BF16, tag="oh")
                nc.sync.dma_start(
                    out=oh[:],
                    in_=oh_d[:, ds((iv + 1) * (2 * CH * BLOC),
                                   2 * CH * BLOC)])
                for sc in range(CH):
                    a, b = (hTA, hTB) if sc % 2 == 0 else (hTB, hTA)
                    step(sc, xgA, a, b)
                produce_xg(oh, 0, xgA)
                for sc in range(CH):
                    a, b = (hTA, hTB) if sc % 2 == 0 else (hTB, hTA)
                    step(sc, xgB, a, b)
                produce_xg(oh, CH * BLOC, xgB)

        with tc.tile_pool(name="psum2", bufs=1, space="PSUM") as psum2:
            fcp = psum2.tile([BLOC, 1], F32, tag="fcp")
            for j in range(4):
                nc.tensor.matmul(fcp[:], hfin[:, j * 8:(j + 1) * 8],
                                 fcw[:, j:j + 1],
                                 start=(j == 0), stop=(j == 3))
            out_sb = scr.tile([BLOC, 1], F32, tag="osb")
            nc.vector.tensor_tensor(out_sb[:], fcp[:], fcb[:], op=ALU.add)
            nc.sync.dma_start(out=out_d[:], in_=out_sb[:])

    _split_multi_waits(nc)
    return nc


_NC_CACHE = None


def kernel(tokens, lengths, W_ih, W_hh, b_ih, b_hh, fc_w, fc_b):
    global _NC_CACHE
    per_core, order = _host_prep(tokens, lengths, W_ih, W_hh, b_ih, b_hh,
                                 fc_w, fc_b)
    if _NC_CACHE is None:
        _NC_CACHE = _build_nc()
    res = run_bass_kernel_spmd(_NC_CACHE, per_core,
                               core_ids=list(range(N_CORES)))
    # reference returns outputs in sorted (desc length) order; shard ci
    # holds sorted ranks ci*8..ci*8+7, so concatenation is already sorted
    out = np.concatenate([res.results[i]["out"] for i in range(N_CORES)],
                         axis=0).astype(np.float32)
    return out
